# revision 10
# baseline (speedup 1.0000x reference)
"""Trainium2 Bass kernel for nn_BalanceDropLoss (histogram_binning).

Math: bce(x,t) = softplus((1-2t)x) = ln(1+exp(z)), z = +-x.  The loss needs
only five per-class sums: A = #(t=1), T = sum(bce), S1 = sum(t*bce),
EB = sum(easy*bce), TEB = sum(t*easy*bce), where easy <=> (2t-1)x > ln9.
The per-class weighting/combine is a tiny [C]-sized computation on the host
(as in the data-parallel baseline).

Layout: each core takes 5 of the 40 classes; each (class, batch-group) pair
is one SBUF partition row (5 cls x 25 groups = 125 rows).  Within a row the
host orders elements into four fixed-size regions by (t, easy) with neutral
padding (x = +-38 -> exp(z) -> 0 -> contributes ln(1) = 0), so the (1-2t)
sign becomes the activation's free `scale` and every needed sum is a
per-partition region sum.

Device pipeline per pass (per core, ~5.4 MB bf16, one DMA):
  u  = Exp(-+x)                  ScalarE, 1 elem/lane/cyc (the roofline)
  r0 = u+1                       DVE tensor_scalar 4x
  r_{k+1} = r_lo * r_hi  (x6)    DVE tensor_tensor 2x, halving each level
    -> r6 = product of 64 (1+u) terms; ln(r6) = sum of 64 bce values
  ln(r6) via one Newton step in exp space (needs only the Exp table ->
  zero activation-table thrash):
    w0 = bits(r6)*(ln2/128) - C  DVE tensor_scalar on a u16 bitcast view
                                 (Schraudolph log, |err| <= 0.0298),
                                 fused accum_out -> sum(w0)
    c  = Exp(-w0)                ScalarE (tiny)
    sum(r6*c)                    DVE affine_mul_reduce accum (tiny)
  host: region_sum = sum(w0) + sum(r6*c) - n_groups   (Newton update summed;
        quadratic bias ~delta^2/2 <= 5e-4 per 64-group, ~1e-5 relative)
"""

import numpy as np
import ml_dtypes

B_TOTAL = 524288
C = 40
NCORES = 8
CLS_PER_CORE = C // NCORES     # 5
G = 25                         # batch groups per class
P = CLS_PER_CORE * G           # 125 partition rows per core
LN9 = 2.1972245773362196       # easy threshold: (2t-1)x > ln(9)
LN2 = 0.6931471805599453
W0C = 127.0 * LN2 - 0.0298     # Schraudolph log bias (centered)
PAD_POS = 38.0                 # pad for t=1 regions (z=-x -> exp->0)
PAD_NEG = -38.0                # pad for t=0 regions (z=+x -> exp->0)
DEPTH = 6                      # pairwise-product levels (64-elem groups)
BF16 = ml_dtypes.bfloat16


def _round_caps(needed):
    """Region caps (r11, r10, r01, r00): multiples of 64; r00 of 128."""
    r = [-(-int(n) // 64) * 64 for n in needed]
    r[3] = -(-int(needed[3]) // 128) * 128
    return tuple(max(v, 64) for v in r)


def _chunks(caps):
    """Device chunks: (col_offset, length, act_scale). r00 split in two."""
    k11, k10, k01, k00 = caps
    h = k00 // 2
    o = np.cumsum([0, k11, k10, k01, h])
    return [
        (int(o[0]), k11, -1.0),
        (int(o[1]), k10, -1.0),
        (int(o[2]), k01, 1.0),
        (int(o[3]), h, 1.0),
        (int(o[4]), k00 - h, 1.0),
    ]


def _build(caps, repeats=1, bufs_x=2, bufs_mid=2):
    from contextlib import ExitStack

    import concourse.bass as bass  # noqa: F401  (registers engines)
    import concourse.tile as tile
    from concourse import bacc, mybir

    f32 = mybir.dt.float32
    bf16 = mybir.dt.bfloat16
    u16 = mybir.dt.uint16
    Act = mybir.ActivationFunctionType
    Alu = mybir.AluOpType

    chunks = _chunks(caps)
    F = int(sum(caps))
    NCH = len(chunks)
    g64 = [L >> DEPTH for _, L, _ in chunks]       # r6 cols per chunk
    g6o = np.concatenate([[0], np.cumsum(g64)])    # r6buf slot offsets
    NG = int(g6o[-1])

    nc = bacc.Bacc(
        "TRN2", target_bir_lowering=False, debug=False, num_devices=NCORES
    )
    x = nc.dram_tensor("x", [P, F], bf16, kind="ExternalInput").ap()
    out = nc.dram_tensor("out", [P, 2 * NCH], f32, kind="ExternalOutput").ap()

    lmax = max(c[1] for c in chunks)
    with tile.TileContext(nc) as tc, ExitStack() as ctx:
        pool = ctx.enter_context(tc.tile_pool(name="main", bufs=1))
        slots = pool.tile([P, 2 * NCH], f32)
        xbig = [
            pool.tile([P, F], bf16, name=f"xbig{i}", tag=f"xbig{i}")
            for i in range(bufs_x)
        ]
        wss = []
        for i in range(bufs_mid):
            ws = {
                "u": pool.tile([P, lmax], bf16, name=f"u{i}", tag=f"u{i}"),
                "r0": pool.tile([P, lmax], bf16, name=f"r0_{i}", tag=f"r0_{i}"),
            }
            for d in range(1, DEPTH):
                ws[f"r{d}"] = pool.tile(
                    [P, lmax >> d], bf16, name=f"r{d}_{i}", tag=f"r{d}_{i}"
                )
            wss.append(ws)
        r6buf = pool.tile([P, NG], bf16)
        w0buf = pool.tile([P, NG], f32)
        cbuf = pool.tile([P, NG], bf16)
        rcbuf = pool.tile([P, NG], bf16)
        w0c_bias = pool.tile([P, 1], f32)
        nc.vector.memset(w0c_bias[:], W0C)

        k = 0
        for _rep in range(repeats):
            xb = xbig[_rep % bufs_x]
            nc.sync.dma_start(xb[:], x)
            for ci, (off, L, scale) in enumerate(chunks):
                ws = wss[k % bufs_mid]
                k += 1
                nc.scalar.activation(
                    ws["u"][:, :L], xb[:, off : off + L], Act.Exp, scale=scale
                )
                nc.vector.tensor_scalar(
                    ws["r0"][:, :L], ws["u"][:, :L], 1.0, None, op0=Alu.add
                )
                cur = ws["r0"]
                ln = L
                for d in range(DEPTH - 1):
                    ln //= 2
                    nxt = ws[f"r{d + 1}"]
                    nc.vector.tensor_tensor(
                        nxt[:, :ln], cur[:, 0:ln], cur[:, ln : 2 * ln],
                        op=Alu.mult,
                    )
                    cur = nxt
                ln //= 2
                nc.vector.tensor_tensor(
                    r6buf[:, int(g6o[ci]) : int(g6o[ci + 1])],
                    cur[:, 0:ln], cur[:, ln : 2 * ln], op=Alu.mult,
                )
            # Newton-ln over the collected 64-group products.
            # NOTE (hw behavior): with accum_out present, tensor_scalar's
            # second scalar acts as the accumulator seed, not a fused op1 —
            # so compute w0' = bits*ln2/128 only, fold the -W0C shift into
            # the Exp bias, and subtract n*W0C on the host.
            for ci in range(NCH):
                s0, s1 = int(g6o[ci]), int(g6o[ci + 1])
                nc.vector.tensor_scalar(
                    w0buf[:, s0:s1], r6buf[:, s0:s1].bitcast(u16),
                    LN2 / 128.0, 0.0, op0=Alu.mult, op1=Alu.add,
                    accum_out=slots[:, ci : ci + 1],
                )
            nc.scalar.activation(
                cbuf[:], w0buf[:], Act.Exp, scale=-1.0, bias=w0c_bias[:]
            )
            for ci in range(NCH):
                s0, s1 = int(g6o[ci]), int(g6o[ci + 1])
                nc.vector.affine_mul_reduce(
                    out=rcbuf[:, s0:s1], accum_out=slots[:, NCH + ci : NCH + ci + 1],
                    in0=r6buf[:, s0:s1], in1=cbuf[:, s0:s1],
                    scale=1.0, bias=0.0,
                )
        nc.sync.dma_start(out, slots[:])

    nc.compile()
    return nc


_NC_CACHE = {}


def _get_nc(caps, repeats=1):
    key = (caps, repeats)
    if key not in _NC_CACHE:
        _NC_CACHE[key] = _build(caps, repeats=repeats)
    return _NC_CACHE[key]


def _prepare(pred, target):
    """Sort/pad host-side into per-core [P, F] bf16 arrays.

    Returns (xarrs, A, caps): A[c] = per-class positive count, caps = the
    four region sizes actually compiled for.
    """
    pred = np.ascontiguousarray(pred, dtype=np.float32)
    target = np.ascontiguousarray(target, dtype=np.float32)
    B = pred.shape[0]
    gsz = [B // G + (1 if i < B % G else 0) for i in range(G)]
    goff = np.concatenate([[0], np.cumsum(gsz)])

    segs = {}
    A = np.zeros(C, dtype=np.float64)
    needed = np.zeros(4, dtype=np.int64)
    for c in range(C):
        xcol = pred[:, c]
        tcol = target[:, c] > 0.5
        e1 = xcol > LN9
        e0 = xcol < -LN9
        A[c] = np.count_nonzero(tcol)
        m = [tcol & e1, tcol & ~e1, (~tcol) & e0, (~tcol) & ~e0]
        for g in range(G):
            sl = slice(int(goff[g]), int(goff[g + 1]))
            vals = [xcol[sl][mk[sl]] for mk in m]
            segs[(c, g)] = vals
            for ri in range(4):
                needed[ri] = max(needed[ri], len(vals[ri]))

    caps = _round_caps(needed)
    k11, k10, k01, k00 = caps
    F = int(sum(caps))
    off = np.cumsum([0, k11, k10, k01])
    xarrs = []
    for core in range(NCORES):
        arr = np.empty((P, F), dtype=BF16)
        arr[:, : k11 + k10] = BF16(PAD_POS)
        arr[:, k11 + k10 :] = BF16(PAD_NEG)
        for lc in range(CLS_PER_CORE):
            c = core * CLS_PER_CORE + lc
            for g in range(G):
                row = lc * G + g
                vals = segs[(c, g)]
                for ri in range(4):
                    v = vals[ri]
                    arr[row, off[ri] : off[ri] + len(v)] = v.astype(BF16)
        xarrs.append(arr)
    return xarrs, A, caps


def _combine(outs, A, caps, b_total=B_TOTAL):
    """Per-core [P, 10] Newton slot sums -> per-class T/S1/EB/TEB -> loss."""
    chunks = _chunks(caps)
    NCH = len(chunks)
    counts = np.array([L >> DEPTH for _, L, _ in chunks], dtype=np.float64)
    T = np.zeros(C)
    S1 = np.zeros(C)
    EB = np.zeros(C)
    TEB = np.zeros(C)
    for core, o in enumerate(outs):
        o = o.astype(np.float64)
        # slot sum = (sum w0' - n*W0C) + (sum r*c) - n  [Newton update]
        s = (o[:, :NCH] + o[:, NCH:] - counts[None, :] * (1.0 + W0C)).reshape(
            CLS_PER_CORE, G, NCH
        )
        cls = slice(core * CLS_PER_CORE, (core + 1) * CLS_PER_CORE)
        T[cls] += s.sum(axis=(1, 2))
        S1[cls] += (s[..., 0] + s[..., 1]).sum(axis=1)
        EB[cls] += (s[..., 0] + s[..., 2]).sum(axis=1)
        TEB[cls] += s[..., 0].sum(axis=1)
    bal = 0.5 * b_total
    neg = b_total - A
    pos_gt = A >= bal
    n_maj = np.where(pos_gt, A, neg)
    s_maj = np.where(pos_gt, S1, T - S1)
    g_maj = np.where(pos_gt, TEB, EB - TEB)
    n_min = np.where(pos_gt, neg, A)
    s_min = np.where(pos_gt, T - S1, S1)
    w_maj = bal / np.maximum(n_maj, 1.0)
    w_min = (b_total - bal) / np.maximum(n_min, 1.0)
    total = (
        w_maj * (s_maj - g_maj) + np.where(n_min > 0, w_min * s_min, 0.0)
    ).sum()
    return np.float32(total / (b_total * C))


def kernel(pred: np.ndarray, target: np.ndarray) -> np.ndarray:
    from concourse.bass_utils import run_bass_kernel_spmd

    xarrs, A, caps = _prepare(pred, target)
    nc = _get_nc(caps)
    in_maps = [{"x": xarrs[i]} for i in range(NCORES)]
    res = run_bass_kernel_spmd(nc, in_maps, list(range(NCORES)))
    outs = [res.results[i]["out"] for i in range(NCORES)]
    return _combine(outs, A, caps, b_total=pred.shape[0])
